# revision 46
# baseline (speedup 1.0000x reference)
"""GQA attention kernel for Trainium2 (8 NeuronCores).

Sharding: batch x head-group tensor parallel. Core c handles batch (c % 2)
and head group (c // 2): 8 q heads + 2 kv heads of that batch. Each core
computes its partial o-proj output (contraction over its 512 attn features);
the host sums the 4 partials per batch.

Device-side layouts (per core):
  xT   [H=2048 hidden, S=2048 tokens] bf16  (x transposed on host)
  Q^T  [dim, tokens] per head-pair tile [128, S]; pair p holds q heads
       (p, p+4) so that rows 0:64 attend kv head 0 and rows 64:128 attend
       kv head 1 (host reorders Wq / Wo features to match).
  K^T  single [128, S] tile: rows 0:64 = kv head 0, 64:128 = kv head 1.
       Scores are K=64 matmuls on aligned 64-row partition ranges.
  V    [tokens, dim] natural layout with an appended ones-column.
  scores S^T[kv, q] = K^T.T @ Q^T per 128-kv tile, exp'd on ACT; the
       diagonal 128x128 block gets a triangular mask (DVE).
  PV   flipped: out[q 128, d+1] accumulates over kv tiles with the prob
       tile stationary -> causal trimming at q-tile granularity; the
       ones-column yields the softmax denominator per q token (partition),
       normalized with a per-partition broadcast multiply into att2
       [tok, feat]. (PSUM note: matmul start=True zeroes the whole bank,
       so only the first matmul into a shared-bank tile carries it.)
  att2 -> att via DMA XBAR transpose (SBUF->SBUF), feeding o-proj.
  RoPE: rot_half is a fixed 128x128 rotation matmul on PE combined with
       cos/sin tables on DVE. The 1/sqrt(64) scale is folded into Wq.

The exp stream on ACT is slower than the matmuls it feeds, so emission is
software-pipelined at tile granularity: a queue of independent PE work
(o-proj units of earlier blocks, projection chains of later blocks) is
pumped between score tiles, sized by an ACT-minus-PE debt counter, keeping
the in-order PE stream busy while ACT works through the exps.
"""

import os
import numpy as np
import ml_dtypes
from collections import deque
from contextlib import ExitStack

import concourse.bass as bass
import concourse.tile as tile
from concourse import bacc
from concourse import mybir
from concourse import bass_utils

BF16 = mybir.dt.bfloat16
F32 = mybir.dt.float32
BF = ml_dtypes.bfloat16
AF = mybir.ActivationFunctionType
OP = mybir.AluOpType

H = 2048
S = 2048
B = 2
D = 64
QH = 8            # q heads per core
KVH = 2           # kv heads per core
QF = QH * D       # 512 q features per core
KF = KVH * D      # 128 kv features per core
NK = H // 128     # 16 contraction tiles
NT = S // 128     # 16 token tiles
QBS = 512         # q block size
NQB = S // QBS    # 4 q blocks
NPAIR = QF // 128 # 4 q head-pair tiles

# cost-model estimates (ns) used to pace the emission interleave
PE_CYC = 1.0 / 2.4
ACT_CYC = 1.0 / 1.2
ROTFIN_NS = 512 * PE_CYC          # rope rotation matmul
OPROJ_NS = 4 * 512 * PE_CYC       # one o-proj psum chain
PV_NS = 65 * PE_CYC               # one PV accumulation step

_CACHE = {}


def _build_program():
    nc = bacc.Bacc(
        "TRN2",
        target_bir_lowering=False,
        debug=False,
        enable_asserts=False,
        num_devices=8,
    )
    xT = nc.dram_tensor("xT", [H, S], BF16, kind="ExternalInput").ap()
    wqT = nc.dram_tensor("wqT", [H, QF], BF16, kind="ExternalInput").ap()
    wkT = nc.dram_tensor("wkT", [H, KF], BF16, kind="ExternalInput").ap()
    wvT = nc.dram_tensor("wvT", [H, KF], BF16, kind="ExternalInput").ap()
    woT = nc.dram_tensor("woT", [QF, H], BF16, kind="ExternalInput").ap()
    cost = nc.dram_tensor("cost", [128, S], BF16, kind="ExternalInput").ap()
    sint = nc.dram_tensor("sint", [128, S], BF16, kind="ExternalInput").ap()
    rotT = nc.dram_tensor("rotT", [128, 128], BF16, kind="ExternalInput").ap()
    maskd = nc.dram_tensor("maskd", [128, 128], BF16, kind="ExternalInput").ap()
    out = nc.dram_tensor("out", [S, H], BF16, kind="ExternalOutput").ap()
    dbg = {}
    if os.environ.get("KERNEL_DEBUG"):
        for nm in ("d_qt0", "d_kt", "d_att0", "d_att1"):
            dbg[nm] = nc.dram_tensor(nm, [128, S], BF16, kind="ExternalOutput").ap()
        dbg["d_va0"] = nc.dram_tensor("d_va0", [128, NT * (D + 1)], BF16, kind="ExternalOutput").ap()

    with tile.TileContext(nc) as tc:
        with ExitStack() as ctx:
            E = ctx.enter_context
            persist = E(tc.tile_pool(name="persist", bufs=1))
            psS = E(tc.tile_pool(name="psS", bufs=2, space="PSUM"))
            psP = E(tc.tile_pool(name="psP", bufs=1, space="PSUM"))
            psPV = E(tc.tile_pool(name="psPV", bufs=1, space="PSUM"))
            psO = E(tc.tile_pool(name="psO", bufs=2, space="PSUM"))
            wk = E(tc.tile_pool(name="wk", bufs=2))
            wkpr = E(tc.tile_pool(name="wkpr", bufs=17))
            wk2 = E(tc.tile_pool(name="wk2", bufs=2))

            # ---------------- constant loads ----------------
            # SP queue: wk/xT interleaved so the first K-proj chain can ride
            # the load wave. ACT queue (second HWDGE): everything else.
            wq_sb = []
            wk_sb = []
            wv_sb = []
            xT_sb = []
            for k in range(NK):
                q = nc.sync if k % 2 == 0 else nc.scalar
                tk = persist.tile([128, KF], BF16, tag=f"wk{k}", name=f"wk{k}")
                q.dma_start(tk[:], wkT[k * 128:(k + 1) * 128, :])
                wk_sb.append(tk)
                t = persist.tile([128, S], BF16, tag=f"xT{k}", name=f"xT{k}")
                q.dma_start(t[:], xT[k * 128:(k + 1) * 128, :])
                xT_sb.append(t)
            for k in range(NK):
                tv = persist.tile([128, KF], BF16, tag=f"wv{k}", name=f"wv{k}")
                nc.scalar.dma_start(tv[:], wvT[k * 128:(k + 1) * 128, :])
                wv_sb.append(tv)
            for k in range(NK):
                tq = persist.tile([128, QF], BF16, tag=f"wq{k}", name=f"wq{k}")
                nc.scalar.dma_start(tq[:], wqT[k * 128:(k + 1) * 128, :])
                wq_sb.append(tq)
            rt = persist.tile([128, 128], BF16, tag="rt")
            nc.scalar.dma_start(rt[:], rotT[:, :])
            cs = persist.tile([128, S], BF16, tag="cs")
            nc.scalar.dma_start(cs[:], cost[:, :])
            sn = persist.tile([128, S], BF16, tag="sn")
            nc.scalar.dma_start(sn[:], sint[:, :])
            msk = persist.tile([128, 128], BF16, tag="msk")
            nc.scalar.dma_start(msk[:], maskd[:, :])
            wo_sb = []
            for p in range(NPAIR):
                t = persist.tile([128, H], BF16, tag=f"wo{p}", name=f"wo{p}")
                nc.scalar.dma_start(t[:], woT[p * 128:(p + 1) * 128, :])
                wo_sb.append(t)

            # ---------------- persistent activation tiles ----------------
            qt_sb = [persist.tile([128, S], BF16, tag=f"qt{p}", name=f"qt{p}") for p in range(NPAIR)]
            kt = persist.tile([128, S], BF16, tag="kt")
            va = [persist.tile([128, NT, D + 1], BF16, tag=f"va{v}", name=f"va{v}") for v in (0, 1)]
            att = [persist.tile([128, S], BF16, tag=f"att{p}", name=f"att{p}") for p in range(NPAIR)]

            nc.vector.memset(va[0][:, :, D:D + 1], 1.0)
            nc.vector.memset(va[1][:, :, D:D + 1], 1.0)

            tbc = lambda tb: slice(tb * QBS, (tb + 1) * QBS)

            def rope_finish(ps, raw, tb, outs):
                rp = psO.tile([128, QBS], F32, tag="op")
                nc.tensor.matmul(rp[:], lhsT=rt[:], rhs=raw[:], start=True, stop=True)
                t1 = wk.tile([128, QBS], BF16, tag="rope_t1")
                nc.vector.tensor_tensor(out=t1[:], in0=rp[:], in1=sn[:, tbc(tb)], op=OP.mult)
                t2 = wk.tile([128, QBS], BF16, tag="rope_t2")
                nc.vector.tensor_tensor(out=t2[:], in0=raw[:], in1=cs[:, tbc(tb)], op=OP.mult)
                for rows, out_ap in outs:
                    nc.vector.tensor_tensor(
                        out=out_ap, in0=t1[rows, :], in1=t2[rows, :], op=OP.add)

            SUB = 4           # matmuls per chain sub-unit
            SUB_NS = SUB * 512 * PE_CYC

            def proj_units(tb):
                """Projection work for token block tb as fine-grained
                (pe_ns, closure) units: 16-step psum chains are split into
                SUB-sized pieces so fills can slot between score pairs."""
                units = []
                stk = {}
                def kpart(k0, tb=tb, stk=stk):
                    if k0 == 0:
                        stk['ps'] = psP.tile([128, QBS], F32, tag="ps", name="kp")
                    for k in range(k0, k0 + SUB):
                        nc.tensor.matmul(
                            stk['ps'][:], lhsT=wk_sb[k][:],
                            rhs=xT_sb[k][:, tbc(tb)],
                            start=(k == 0), stop=(k == NK - 1))
                    if k0 + SUB == NK:
                        raw = wk.tile([128, QBS], BF16, tag="rope_raw")
                        nc.vector.tensor_copy(out=raw[:], in_=stk['ps'][:])
                        stk['raw'] = raw
                for k0 in range(0, NK, SUB):
                    units.append((SUB_NS, lambda k0=k0: kpart(k0)))
                units.append((ROTFIN_NS, lambda tb=tb, stk=stk: rope_finish(
                    None, stk['raw'], tb,
                    [(slice(0, 64), kt[0:64, tbc(tb)]),
                     (slice(64, 128), kt[64:128, tbc(tb)])])))

                for t in range(4 * tb, 4 * tb + 4):
                    stv = {}
                    def vpart(k0, t=t, stv=stv):
                        if k0 == 0:
                            stv['ps'] = psP.tile([128, QBS], F32, tag="ps", name="vp")
                        for k in range(k0, k0 + SUB):
                            nc.tensor.matmul(
                                stv['ps'][:, 0:KF],
                                lhsT=xT_sb[k][:, t * 128:(t + 1) * 128],
                                rhs=wv_sb[k][:],
                                start=(k == 0), stop=(k == NK - 1))
                        if k0 + SUB == NK:
                            for v in (0, 1):
                                nc.vector.tensor_copy(
                                    out=va[v][:, t, 0:D],
                                    in_=stv['ps'][:, v * D:(v + 1) * D])
                    for k0 in range(0, NK, SUB):
                        units.append((SUB_NS, lambda k0=k0, vp=vpart: vp(k0)))

                for p in range(NPAIR):
                    stq = {}
                    def qpart(k0, p=p, tb=tb, stq=stq):
                        if k0 == 0:
                            stq['ps'] = psP.tile([128, QBS], F32, tag="ps", name="qp")
                        for k in range(k0, k0 + SUB):
                            nc.tensor.matmul(
                                stq['ps'][:],
                                lhsT=wq_sb[k][:, p * 128:(p + 1) * 128],
                                rhs=xT_sb[k][:, tbc(tb)],
                                start=(k == 0), stop=(k == NK - 1))
                        if k0 + SUB == NK:
                            raw = wk.tile([128, QBS], BF16, tag="rope_raw")
                            nc.vector.tensor_copy(out=raw[:], in_=stq['ps'][:])
                            stq['raw'] = raw
                    for k0 in range(0, NK, SUB):
                        units.append((SUB_NS, lambda k0=k0, qp=qpart: qp(k0)))
                    units.append((ROTFIN_NS, lambda p=p, tb=tb, stq=stq:
                                  rope_finish(
                                      None, stq['raw'], tb,
                                      [(slice(0, 128), qt_sb[p][:, tbc(tb)])])))
                return units

            def oproj_units(qb, tail=False):
                units = []

                def unit(t, n):
                    op_ps = psO.tile([128, 512], F32, tag="op")
                    for p in range(NPAIR):
                        nc.tensor.matmul(
                            op_ps[:], lhsT=att[p][:, t * 128:(t + 1) * 128],
                            rhs=wo_sb[p][:, n * 512:(n + 1) * 512],
                            start=(p == 0), stop=(p == NPAIR - 1))
                    o_sb = wk.tile([128, 512], BF16, tag="osb", bufs=3)
                    # in the tail ACT has no exps left; let the scheduler
                    # spread the drains across idle engines
                    eng = nc.any if tail else nc.vector
                    eng.tensor_copy(out=o_sb[:], in_=op_ps[:])
                    nc.sync.dma_start(
                        out[t * 128:(t + 1) * 128, n * 512:(n + 1) * 512],
                        o_sb[:])
                for t in range(4 * qb, 4 * qb + 4):
                    for n in range(4):
                        units.append((OPROJ_NS, lambda t=t, n=n: unit(t, n)))
                return units

            # fill queues: projq must drain before its block's attention;
            # oprojq can roll over into later blocks.
            projq = deque()
            oprojq = deque()
            debt = [0.0]

            def pump():
                while debt[0] > 0 and (projq or oprojq):
                    q = projq if projq else oprojq
                    pe_ns, cl = q.popleft()
                    cl()
                    # clamp banked PE credit: the sc psum ring is only 3 deep,
                    # so surplus beyond ~2 exp drains is wasted on a stall
                    debt[0] = max(debt[0] - pe_ns, -600.0)

            def emit_attn_head(qb, hh, att2):
                p = hh & 3
                half = hh >> 2
                hr = slice(64 * half, 64 * half + 64)
                nkv = 4 * qb + 4
                prs = []
                # score phase: kv-tile pairs share one 2-bank psum slot and
                # one exp spanning both (start=True zeroes each bank, so the
                # unwritten diagonal region reads exp(0), never consumed).
                # PE streams score matmuls; ACT exps lag; fills are pumped
                # between pairs to keep the in-order PE stream busy.
                for kvp in range(nkv // 2):
                    sc = psS.tile([128, 2 * QBS], F32, tag="sc")
                    pr2 = wkpr.tile([128, 2 * QBS], BF16, tag="pr")
                    c00 = 0
                    for sub in (0, 1):
                        kv = 2 * kvp + sub
                        j = kv - 4 * qb
                        c0 = max(0, 128 * j)
                        if sub == 0:
                            c00 = c0
                        base = sub * QBS
                        nc.tensor.matmul(
                            sc[:, base + c0:base + QBS],
                            lhsT=kt[hr, kv * 128:(kv + 1) * 128],
                            rhs=qt_sb[p][hr, qb * QBS + c0:(qb + 1) * QBS],
                            start=True, stop=True)
                    c01 = max(0, 128 * (2 * kvp + 1 - 4 * qb))
                    if c01 > 256:
                        # large zeroed gap between the two valid spans:
                        # two exps cost less than exp-ing the gap
                        nc.scalar.activation(
                            pr2[:, c00:QBS], sc[:, c00:QBS], AF.Exp)
                        nc.scalar.activation(
                            pr2[:, QBS + c01:2 * QBS],
                            sc[:, QBS + c01:2 * QBS], AF.Exp)
                    else:
                        nc.scalar.activation(
                            pr2[:, c00:2 * QBS], sc[:, c00:2 * QBS], AF.Exp)
                    for sub in (0, 1):
                        kv = 2 * kvp + sub
                        j = kv - 4 * qb
                        c0 = max(0, 128 * j)
                        base = sub * QBS
                        prm = None
                        if j >= 0:
                            # mask the diagonal 128x128 block (kv>q -> 0)
                            prm = wkpr.tile([128, 128], BF16, tag="prm", bufs=10)
                            nc.vector.tensor_tensor(
                                out=prm[:], in0=pr2[:, base + c0:base + c0 + 128],
                                in1=msk[:], op=OP.mult)
                        prs.append((pr2, base, prm, j))
                    w2 = 2 * QBS - c00
                    debt[0] += (w2 * ACT_CYC + 400.0) - (w2 * PE_CYC)
                    pump()
                return prs

            def emit_pv(qb, hh, prs, att2):
                p = hh & 3
                half = hh >> 2
                nkv = 4 * qb + 4
                pv = psPV.tile([128, 4, D + 1], F32, tag="pv")
                for kv in range(nkv):
                    pr2, base, prm, j = prs[kv]
                    for qsub in range(max(0, j), 4):
                        lhs = prm[:] if qsub == j else \
                            pr2[:, base + qsub * 128:base + (qsub + 1) * 128]
                        # start=True zeroes the whole psum bank, so only the
                        # first matmul into this tile may carry it
                        nc.tensor.matmul(
                            pv[:, qsub, :], lhsT=lhs,
                            rhs=va[half][:, kv, :],
                            start=(kv == 0 and qsub == 0),
                            stop=(kv == 4 * qb + qsub),
                            skip_group_check=True)
                        debt[0] -= PV_NS
                # normalize by the ones-column sums (per q token)
                rec = wk2.tile([128, 4, 1], F32, tag="rec")
                nc.vector.reciprocal(rec[:, :, 0], pv[:, :, D])
                nc.vector.tensor_tensor(
                    out=att2[p][:, :, 64 * half:64 * half + 64],
                    in0=pv[:, :, 0:D],
                    in1=rec[:].broadcast_to([128, 4, D]),
                    op=OP.mult)

            # ---------------- software-pipelined emission ----------------
            for _, u in proj_units(0):
                u()
            for qb in range(NQB):
                att2 = [wk2.tile([128, 4, 128], BF16, tag=f"att2_{p}",
                                 name=f"att2_{p}") for p in range(NPAIR)]
                if qb > 0:
                    oprojq.extend(oproj_units(qb - 1))
                if qb + 1 < NQB:
                    projq.extend(proj_units(qb + 1))
                # one-head software pipeline: pv(h-1) is emitted after
                # scores(h), so its exps are long done when PE reaches it
                pending = None
                for hh in range(QH):
                    prs = emit_attn_head(qb, hh, att2)
                    if pending is not None:
                        emit_pv(qb, pending[0], pending[1], att2)
                    pending = (hh, prs)
                emit_pv(qb, pending[0], pending[1], att2)
                # proj of the next block must complete before its attention
                while projq:
                    _, cl = projq.popleft()
                    cl()
                for pp in range(NPAIR):
                    for qsub in range(4):
                        t = 4 * qb + qsub
                        nc.sync.dma_start_transpose(
                            att[pp][:, t * 128:(t + 1) * 128],
                            att2[pp][:, qsub, :])
            while oprojq:
                _, cl = oprojq.popleft()
                cl()
            for _, cl in oproj_units(NQB - 1, tail=True):
                cl()
            if dbg:
                nc.sync.dma_start(dbg["d_qt0"][:, :], qt_sb[0][:])
                nc.sync.dma_start(dbg["d_kt"][:, :], kt[:])
                nc.sync.dma_start(dbg["d_att0"][:, :], att[0][:])
                nc.sync.dma_start(dbg["d_att1"][:, :], att[1][:])
                nc.sync.dma_start(dbg["d_va0"][:, :], va[0][:, :, :])
    nc.compile()
    return nc


def _host_tables():
    freq = 1.0 / (10000.0 ** (np.arange(0, D, 2, dtype=np.float64) / D))
    t = np.arange(S, dtype=np.float64)
    fr = t[:, None] * freq[None, :]                       # (S, 32)
    emb = np.concatenate([fr, fr], axis=-1)               # (S, 64)
    cos64 = np.cos(emb).T.astype(np.float32)              # (64, S)
    sin64 = np.sin(emb).T.astype(np.float32)
    cos128 = np.concatenate([cos64, cos64], axis=0).astype(BF)
    sin128 = np.concatenate([sin64, sin64], axis=0).astype(BF)
    R = np.zeros((64, 64), np.float32)
    R[np.arange(32), 32 + np.arange(32)] = -1.0
    R[32 + np.arange(32), np.arange(32)] = 1.0
    R128 = np.zeros((128, 128), np.float32)
    R128[:64, :64] = R
    R128[64:, 64:] = R
    rotT = np.ascontiguousarray(R128.T).astype(BF)
    # diagonal-block causal mask: keep q >= kv within a 128x128 block
    r = np.arange(128)[:, None]
    c = np.arange(128)[None, :]
    mask = (r <= c).astype(np.float32).astype(BF)
    return cos128, sin128, rotT, mask


def kernel(x, Wq, Wk, Wv, Wo):
    x = np.asarray(x, np.float32)
    Wq = np.asarray(Wq, np.float32)
    Wk = np.asarray(Wk, np.float32)
    Wv = np.asarray(Wv, np.float32)
    Wo = np.asarray(Wo, np.float32)

    if "nc" not in _CACHE:
        _CACHE["nc"] = _build_program()
    nc = _CACHE["nc"]

    cos128, sin128, rotT, maskb = _host_tables()
    # feature order per pair tile: q heads (p, p+4) -> rows 0:64 face kv
    # head 0, rows 64:128 face kv head 1
    perm = np.concatenate(
        [np.r_[p * D:(p + 1) * D, (p + 4) * D:(p + 5) * D] for p in range(NPAIR)])
    in_maps = []
    for core in range(8):
        g, b = core // 2, core % 2
        wq_g = Wq[QF * g:QF * (g + 1), :][perm, :]
        wo_g = Wo[:, QF * g:QF * (g + 1)][:, perm]
        im = {
            "xT": np.ascontiguousarray(x[b].T).astype(BF),
            "wqT": np.ascontiguousarray((wq_g / 8.0).T).astype(BF),
            "wkT": np.ascontiguousarray(Wk[KF * g:KF * (g + 1), :].T).astype(BF),
            "wvT": np.ascontiguousarray(Wv[KF * g:KF * (g + 1), :].T).astype(BF),
            "woT": np.ascontiguousarray(wo_g.T).astype(BF),
            "cost": cos128,
            "sint": sin128,
            "rotT": rotT,
            "maskd": maskb,
        }
        in_maps.append(im)

    trace = bool(int(os.environ.get("KERNEL_TRACE", "0")))
    res = bass_utils.run_bass_kernel_spmd(
        nc, in_maps, core_ids=list(range(8)), trace=trace)
    _CACHE["last_result"] = res

    out = np.zeros((B, S, H), np.float32)
    for core in range(8):
        g, b = core // 2, core % 2
        out[b] += np.asarray(res.results[core]["out"], np.float32)
    return out


# revision 47
# speedup vs baseline: 1.4000x; 1.4000x over previous
"""GQA attention kernel for Trainium2 (8 NeuronCores).

Sharding: batch x head-group tensor parallel. Core c handles batch (c % 2)
and head group (c // 2): 8 q heads + 2 kv heads of that batch. Each core
computes its partial o-proj output (contraction over its 512 attn features);
the host sums the 4 partials per batch.

Device-side layouts (per core):
  xT   [H=2048 hidden, S=2048 tokens] bf16  (x transposed on host)
  Q^T  [dim, tokens] per head-pair tile [128, S]; pair p holds q heads
       (p, p+4) so that rows 0:64 attend kv head 0 and rows 64:128 attend
       kv head 1 (host reorders Wq / Wo features to match).
  K^T  single [128, S] tile: rows 0:64 = kv head 0, 64:128 = kv head 1.
       Scores are K=64 matmuls on aligned 64-row partition ranges.
  V    [tokens, dim] natural layout with an appended ones-column.
  scores S^T[kv, q] = K^T.T @ Q^T per 128-kv tile, exp'd on ACT; the
       diagonal 128x128 block gets a triangular mask (DVE).
  PV   flipped: out[q 128, d+1] accumulates over kv tiles with the prob
       tile stationary -> causal trimming at q-tile granularity; the
       ones-column yields the softmax denominator per q token (partition),
       normalized with a per-partition broadcast multiply into att2
       [tok, feat]. (PSUM note: matmul start=True zeroes the whole bank,
       so only the first matmul into a shared-bank tile carries it.)
  att2 -> att via DMA XBAR transpose (SBUF->SBUF), feeding o-proj.
  RoPE: rot_half is a fixed 128x128 rotation matmul on PE combined with
       cos/sin tables on DVE. The 1/sqrt(64) scale is folded into Wq.

The exp stream on ACT is slower than the matmuls it feeds, so emission is
software-pipelined at tile granularity: a queue of independent PE work
(o-proj units of earlier blocks, projection chains of later blocks) is
pumped between score tiles, sized by an ACT-minus-PE debt counter, keeping
the in-order PE stream busy while ACT works through the exps.
"""

import os
import numpy as np
import ml_dtypes
from collections import deque
from contextlib import ExitStack

import concourse.bass as bass
import concourse.tile as tile
from concourse import bacc
from concourse import mybir
from concourse import bass_utils

BF16 = mybir.dt.bfloat16
F32 = mybir.dt.float32
BF = ml_dtypes.bfloat16
AF = mybir.ActivationFunctionType
OP = mybir.AluOpType

H = 2048
S = 2048
B = 2
D = 64
QH = 8            # q heads per core
KVH = 2           # kv heads per core
QF = QH * D       # 512 q features per core
KF = KVH * D      # 128 kv features per core
NK = H // 128     # 16 contraction tiles
NT = S // 128     # 16 token tiles
QBS = 512         # q block size
NQB = S // QBS    # 4 q blocks
NPAIR = QF // 128 # 4 q head-pair tiles

# cost-model estimates (ns) used to pace the emission interleave
PE_CYC = 1.0 / 2.4
ACT_CYC = 1.0 / 1.2
ROTFIN_NS = 512 * PE_CYC          # rope rotation matmul
OPROJ_NS = 4 * 512 * PE_CYC       # one o-proj psum chain
PV_NS = 65 * PE_CYC               # one PV accumulation step

_CACHE = {}


def _build_program():
    nc = bacc.Bacc(
        "TRN2",
        target_bir_lowering=False,
        debug=False,
        enable_asserts=False,
        num_devices=8,
    )
    xT = nc.dram_tensor("xT", [H, S], BF16, kind="ExternalInput").ap()
    wqT = nc.dram_tensor("wqT", [H, QF], BF16, kind="ExternalInput").ap()
    wkT = nc.dram_tensor("wkT", [H, KF], BF16, kind="ExternalInput").ap()
    wvT = nc.dram_tensor("wvT", [H, KF], BF16, kind="ExternalInput").ap()
    woT = nc.dram_tensor("woT", [QF, H], BF16, kind="ExternalInput").ap()
    cost = nc.dram_tensor("cost", [128, S], BF16, kind="ExternalInput").ap()
    sint = nc.dram_tensor("sint", [128, S], BF16, kind="ExternalInput").ap()
    rotT = nc.dram_tensor("rotT", [128, 128], BF16, kind="ExternalInput").ap()
    maskd = nc.dram_tensor("maskd", [128, 128], BF16, kind="ExternalInput").ap()
    out = nc.dram_tensor("out", [S, H], BF16, kind="ExternalOutput").ap()
    dbg = {}
    if os.environ.get("KERNEL_DEBUG"):
        for nm in ("d_qt0", "d_kt", "d_att0", "d_att1"):
            dbg[nm] = nc.dram_tensor(nm, [128, S], BF16, kind="ExternalOutput").ap()
        dbg["d_va0"] = nc.dram_tensor("d_va0", [128, NT * (D + 1)], BF16, kind="ExternalOutput").ap()

    with tile.TileContext(nc) as tc:
        with ExitStack() as ctx:
            E = ctx.enter_context
            persist = E(tc.tile_pool(name="persist", bufs=1))
            psS = E(tc.tile_pool(name="psS", bufs=2, space="PSUM"))
            psP = E(tc.tile_pool(name="psP", bufs=1, space="PSUM"))
            psPV = E(tc.tile_pool(name="psPV", bufs=1, space="PSUM"))
            psO = E(tc.tile_pool(name="psO", bufs=2, space="PSUM"))
            wk = E(tc.tile_pool(name="wk", bufs=2))
            wkpr = E(tc.tile_pool(name="wkpr", bufs=17))
            wk2 = E(tc.tile_pool(name="wk2", bufs=2))

            # ---------------- constant loads ----------------
            # SP queue: wk/xT interleaved so the first K-proj chain can ride
            # the load wave. ACT queue (second HWDGE): everything else.
            wq_sb = []
            wk_sb = []
            wv_sb = []
            xT_sb = []
            for k in range(NK):
                q = nc.sync if k % 2 == 0 else nc.scalar
                tk = persist.tile([128, KF], BF16, tag=f"wk{k}", name=f"wk{k}")
                q.dma_start(tk[:], wkT[k * 128:(k + 1) * 128, :])
                wk_sb.append(tk)
                t = persist.tile([128, S], BF16, tag=f"xT{k}", name=f"xT{k}")
                q.dma_start(t[:], xT[k * 128:(k + 1) * 128, :])
                xT_sb.append(t)
            for k in range(NK):
                tv = persist.tile([128, KF], BF16, tag=f"wv{k}", name=f"wv{k}")
                nc.scalar.dma_start(tv[:], wvT[k * 128:(k + 1) * 128, :])
                wv_sb.append(tv)
            for k in range(NK):
                tq = persist.tile([128, QF], BF16, tag=f"wq{k}", name=f"wq{k}")
                nc.scalar.dma_start(tq[:], wqT[k * 128:(k + 1) * 128, :])
                wq_sb.append(tq)
            rt = persist.tile([128, 128], BF16, tag="rt")
            nc.scalar.dma_start(rt[:], rotT[:, :])
            cs = persist.tile([128, S], BF16, tag="cs")
            nc.scalar.dma_start(cs[:], cost[:, :])
            sn = persist.tile([128, S], BF16, tag="sn")
            nc.scalar.dma_start(sn[:], sint[:, :])
            msk = persist.tile([128, 128], BF16, tag="msk")
            nc.scalar.dma_start(msk[:], maskd[:, :])
            wo_sb = []
            for p in range(NPAIR):
                t = persist.tile([128, H], BF16, tag=f"wo{p}", name=f"wo{p}")
                nc.scalar.dma_start(t[:], woT[p * 128:(p + 1) * 128, :])
                wo_sb.append(t)

            # ---------------- persistent activation tiles ----------------
            qt_sb = [persist.tile([128, S], BF16, tag=f"qt{p}", name=f"qt{p}") for p in range(NPAIR)]
            kt = persist.tile([128, S], BF16, tag="kt")
            va = [persist.tile([128, NT, D + 1], BF16, tag=f"va{v}", name=f"va{v}") for v in (0, 1)]
            att = [persist.tile([128, S], BF16, tag=f"att{p}", name=f"att{p}") for p in range(NPAIR)]

            nc.vector.memset(va[0][:, :, D:D + 1], 1.0)
            nc.vector.memset(va[1][:, :, D:D + 1], 1.0)

            tbc = lambda tb: slice(tb * QBS, (tb + 1) * QBS)

            def rope_finish(ps, raw, tb, outs):
                rp = psO.tile([128, QBS], F32, tag="op")
                nc.tensor.matmul(rp[:], lhsT=rt[:], rhs=raw[:], start=True, stop=True)
                t1 = wk.tile([128, QBS], BF16, tag="rope_t1")
                nc.vector.tensor_tensor(out=t1[:], in0=rp[:], in1=sn[:, tbc(tb)], op=OP.mult)
                t2 = wk.tile([128, QBS], BF16, tag="rope_t2")
                nc.vector.tensor_tensor(out=t2[:], in0=raw[:], in1=cs[:, tbc(tb)], op=OP.mult)
                for rows, out_ap in outs:
                    nc.vector.tensor_tensor(
                        out=out_ap, in0=t1[rows, :], in1=t2[rows, :], op=OP.add)

            SUB = 4           # matmuls per chain sub-unit
            SUB_NS = SUB * 512 * PE_CYC

            def proj_units(tb):
                """Projection work for token block tb as fine-grained
                (pe_ns, closure) units: 16-step psum chains are split into
                SUB-sized pieces so fills can slot between score pairs."""
                units = []
                stk = {}
                def kpart(k0, tb=tb, stk=stk):
                    if k0 == 0:
                        stk['ps'] = psP.tile([128, QBS], F32, tag="ps", name="kp")
                    for k in range(k0, k0 + SUB):
                        nc.tensor.matmul(
                            stk['ps'][:], lhsT=wk_sb[k][:],
                            rhs=xT_sb[k][:, tbc(tb)],
                            start=(k == 0), stop=(k == NK - 1))
                    if k0 + SUB == NK:
                        raw = wk.tile([128, QBS], BF16, tag="rope_raw")
                        nc.vector.tensor_copy(out=raw[:], in_=stk['ps'][:])
                        stk['raw'] = raw
                for k0 in range(0, NK, SUB):
                    units.append((SUB_NS, lambda k0=k0: kpart(k0)))
                units.append((ROTFIN_NS, lambda tb=tb, stk=stk: rope_finish(
                    None, stk['raw'], tb,
                    [(slice(0, 64), kt[0:64, tbc(tb)]),
                     (slice(64, 128), kt[64:128, tbc(tb)])])))

                for t in range(4 * tb, 4 * tb + 4):
                    stv = {}
                    def vpart(k0, t=t, stv=stv):
                        if k0 == 0:
                            stv['ps'] = psP.tile([128, QBS], F32, tag="ps", name="vp")
                        for k in range(k0, k0 + SUB):
                            nc.tensor.matmul(
                                stv['ps'][:, 0:KF],
                                lhsT=xT_sb[k][:, t * 128:(t + 1) * 128],
                                rhs=wv_sb[k][:],
                                start=(k == 0), stop=(k == NK - 1))
                        if k0 + SUB == NK:
                            for v in (0, 1):
                                nc.vector.tensor_copy(
                                    out=va[v][:, t, 0:D],
                                    in_=stv['ps'][:, v * D:(v + 1) * D])
                    for k0 in range(0, NK, SUB):
                        units.append((SUB_NS, lambda k0=k0, vp=vpart: vp(k0)))

                for p in range(NPAIR):
                    stq = {}
                    def qpart(k0, p=p, tb=tb, stq=stq):
                        if k0 == 0:
                            stq['ps'] = psP.tile([128, QBS], F32, tag="ps", name="qp")
                        for k in range(k0, k0 + SUB):
                            nc.tensor.matmul(
                                stq['ps'][:],
                                lhsT=wq_sb[k][:, p * 128:(p + 1) * 128],
                                rhs=xT_sb[k][:, tbc(tb)],
                                start=(k == 0), stop=(k == NK - 1))
                        if k0 + SUB == NK:
                            raw = wk.tile([128, QBS], BF16, tag="rope_raw")
                            nc.vector.tensor_copy(out=raw[:], in_=stq['ps'][:])
                            stq['raw'] = raw
                    for k0 in range(0, NK, SUB):
                        units.append((SUB_NS, lambda k0=k0, qp=qpart: qp(k0)))
                    units.append((ROTFIN_NS, lambda p=p, tb=tb, stq=stq:
                                  rope_finish(
                                      None, stq['raw'], tb,
                                      [(slice(0, 128), qt_sb[p][:, tbc(tb)])])))
                return units

            def oproj_units(qb, tail=False):
                units = []

                def unit(t, n):
                    op_ps = psO.tile([128, 512], F32, tag="op")
                    for p in range(NPAIR):
                        nc.tensor.matmul(
                            op_ps[:], lhsT=att[p][:, t * 128:(t + 1) * 128],
                            rhs=wo_sb[p][:, n * 512:(n + 1) * 512],
                            start=(p == 0), stop=(p == NPAIR - 1))
                    o_sb = wk.tile([128, 512], BF16, tag="osb", bufs=3)
                    # in the tail ACT has no exps left; let the scheduler
                    # spread the drains across idle engines
                    eng = nc.any if tail else nc.vector
                    eng.tensor_copy(out=o_sb[:], in_=op_ps[:])
                    nc.sync.dma_start(
                        out[t * 128:(t + 1) * 128, n * 512:(n + 1) * 512],
                        o_sb[:])
                for t in range(4 * qb, 4 * qb + 4):
                    for n in range(4):
                        units.append((OPROJ_NS, lambda t=t, n=n: unit(t, n)))
                return units

            # fill queues: projq must drain before its block's attention;
            # oprojq can roll over into later blocks.
            projq = deque()
            oprojq = deque()
            debt = [0.0]

            def pump():
                while debt[0] > 0 and (projq or oprojq):
                    q = projq if projq else oprojq
                    pe_ns, cl = q.popleft()
                    cl()
                    # clamp banked PE credit: the sc psum ring is only 2
                    # pair-slots deep, so surplus beyond ~1 exp drain is
                    # wasted on a ring stall anyway
                    debt[0] = max(debt[0] - pe_ns, -600.0)

            def emit_attn_head(qb, hh, att2):
                p = hh & 3
                half = hh >> 2
                hr = slice(64 * half, 64 * half + 64)
                nkv = 4 * qb + 4
                prs = []
                # score phase: kv-tile pairs share one 2-bank psum slot and
                # one exp spanning both (start=True zeroes each bank, so the
                # unwritten diagonal region reads exp(0), never consumed).
                # PE streams score matmuls; ACT exps lag; fills are pumped
                # between pairs to keep the in-order PE stream busy.
                for kvp in range(nkv // 2):
                    sc = psS.tile([128, 2 * QBS], F32, tag="sc")
                    pr2 = wkpr.tile([128, 2 * QBS], BF16, tag="pr")
                    c00 = 0
                    for sub in (0, 1):
                        kv = 2 * kvp + sub
                        j = kv - 4 * qb
                        c0 = max(0, 128 * j)
                        if sub == 0:
                            c00 = c0
                        base = sub * QBS
                        nc.tensor.matmul(
                            sc[:, base + c0:base + QBS],
                            lhsT=kt[hr, kv * 128:(kv + 1) * 128],
                            rhs=qt_sb[p][hr, qb * QBS + c0:(qb + 1) * QBS],
                            start=True, stop=True)
                    c01 = max(0, 128 * (2 * kvp + 1 - 4 * qb))
                    if c01 > 256:
                        # large zeroed gap between the two valid spans:
                        # two exps cost less than exp-ing the gap
                        nc.scalar.activation(
                            pr2[:, c00:QBS], sc[:, c00:QBS], AF.Exp)
                        nc.scalar.activation(
                            pr2[:, QBS + c01:2 * QBS],
                            sc[:, QBS + c01:2 * QBS], AF.Exp)
                    else:
                        nc.scalar.activation(
                            pr2[:, c00:2 * QBS], sc[:, c00:2 * QBS], AF.Exp)
                    for sub in (0, 1):
                        kv = 2 * kvp + sub
                        j = kv - 4 * qb
                        c0 = max(0, 128 * j)
                        base = sub * QBS
                        prm = None
                        if j >= 0:
                            # mask the diagonal 128x128 block (kv>q -> 0)
                            prm = wkpr.tile([128, 128], BF16, tag="prm", bufs=10)
                            nc.vector.tensor_tensor(
                                out=prm[:], in0=pr2[:, base + c0:base + c0 + 128],
                                in1=msk[:], op=OP.mult)
                        prs.append((pr2, base, prm, j))
                    w2 = 2 * QBS - c00
                    debt[0] += (w2 * ACT_CYC + 400.0) - (w2 * PE_CYC)
                    pump()
                return prs

            def emit_pv(qb, hh, prs, att2):
                p = hh & 3
                half = hh >> 2
                nkv = 4 * qb + 4
                pv = psPV.tile([128, 4, D + 1], F32, tag="pv")
                for kv in range(nkv):
                    pr2, base, prm, j = prs[kv]
                    for qsub in range(max(0, j), 4):
                        lhs = prm[:] if qsub == j else \
                            pr2[:, base + qsub * 128:base + (qsub + 1) * 128]
                        # start=True zeroes the whole psum bank, so only the
                        # first matmul into this tile may carry it
                        nc.tensor.matmul(
                            pv[:, qsub, :], lhsT=lhs,
                            rhs=va[half][:, kv, :],
                            start=(kv == 0 and qsub == 0),
                            stop=(kv == 4 * qb + qsub),
                            skip_group_check=True)
                        debt[0] -= PV_NS
                # normalize by the ones-column sums (per q token)
                rec = wk2.tile([128, 4, 1], F32, tag="rec")
                nc.vector.reciprocal(rec[:, :, 0], pv[:, :, D])
                nc.vector.tensor_tensor(
                    out=att2[p][:, :, 64 * half:64 * half + 64],
                    in0=pv[:, :, 0:D],
                    in1=rec[:].broadcast_to([128, 4, D]),
                    op=OP.mult)

            # ---------------- software-pipelined emission ----------------
            for _, u in proj_units(0):
                u()
            for qb in range(NQB):
                att2 = [wk2.tile([128, 4, 128], BF16, tag=f"att2_{p}",
                                 name=f"att2_{p}") for p in range(NPAIR)]
                if qb > 0:
                    oprojq.extend(oproj_units(qb - 1))
                if qb + 1 < NQB:
                    projq.extend(proj_units(qb + 1))
                # one-head software pipeline: pv(h-1) is emitted after
                # scores(h), so its exps are long done when PE reaches it
                pending = None
                for hh in range(QH):
                    prs = emit_attn_head(qb, hh, att2)
                    if pending is not None:
                        emit_pv(qb, pending[0], pending[1], att2)
                    pending = (hh, prs)
                emit_pv(qb, pending[0], pending[1], att2)
                # proj of the next block must complete before its attention
                while projq:
                    _, cl = projq.popleft()
                    cl()
                for pp in range(NPAIR):
                    for qsub in range(4):
                        t = 4 * qb + qsub
                        nc.sync.dma_start_transpose(
                            att[pp][:, t * 128:(t + 1) * 128],
                            att2[pp][:, qsub, :])
            while oprojq:
                _, cl = oprojq.popleft()
                cl()
            for _, cl in oproj_units(NQB - 1, tail=True):
                cl()
            if dbg:
                nc.sync.dma_start(dbg["d_qt0"][:, :], qt_sb[0][:])
                nc.sync.dma_start(dbg["d_kt"][:, :], kt[:])
                nc.sync.dma_start(dbg["d_att0"][:, :], att[0][:])
                nc.sync.dma_start(dbg["d_att1"][:, :], att[1][:])
                nc.sync.dma_start(dbg["d_va0"][:, :], va[0][:, :, :])
    nc.compile()
    return nc


def _host_tables():
    freq = 1.0 / (10000.0 ** (np.arange(0, D, 2, dtype=np.float64) / D))
    t = np.arange(S, dtype=np.float64)
    fr = t[:, None] * freq[None, :]                       # (S, 32)
    emb = np.concatenate([fr, fr], axis=-1)               # (S, 64)
    cos64 = np.cos(emb).T.astype(np.float32)              # (64, S)
    sin64 = np.sin(emb).T.astype(np.float32)
    cos128 = np.concatenate([cos64, cos64], axis=0).astype(BF)
    sin128 = np.concatenate([sin64, sin64], axis=0).astype(BF)
    R = np.zeros((64, 64), np.float32)
    R[np.arange(32), 32 + np.arange(32)] = -1.0
    R[32 + np.arange(32), np.arange(32)] = 1.0
    R128 = np.zeros((128, 128), np.float32)
    R128[:64, :64] = R
    R128[64:, 64:] = R
    rotT = np.ascontiguousarray(R128.T).astype(BF)
    # diagonal-block causal mask: keep q >= kv within a 128x128 block
    r = np.arange(128)[:, None]
    c = np.arange(128)[None, :]
    mask = (r <= c).astype(np.float32).astype(BF)
    return cos128, sin128, rotT, mask


def kernel(x, Wq, Wk, Wv, Wo):
    x = np.asarray(x, np.float32)
    Wq = np.asarray(Wq, np.float32)
    Wk = np.asarray(Wk, np.float32)
    Wv = np.asarray(Wv, np.float32)
    Wo = np.asarray(Wo, np.float32)

    if "nc" not in _CACHE:
        _CACHE["nc"] = _build_program()
    nc = _CACHE["nc"]

    cos128, sin128, rotT, maskb = _host_tables()
    # feature order per pair tile: q heads (p, p+4) -> rows 0:64 face kv
    # head 0, rows 64:128 face kv head 1
    perm = np.concatenate(
        [np.r_[p * D:(p + 1) * D, (p + 4) * D:(p + 5) * D] for p in range(NPAIR)])
    in_maps = []
    for core in range(8):
        g, b = core // 2, core % 2
        wq_g = Wq[QF * g:QF * (g + 1), :][perm, :]
        wo_g = Wo[:, QF * g:QF * (g + 1)][:, perm]
        im = {
            "xT": np.ascontiguousarray(x[b].T).astype(BF),
            "wqT": np.ascontiguousarray((wq_g / 8.0).T).astype(BF),
            "wkT": np.ascontiguousarray(Wk[KF * g:KF * (g + 1), :].T).astype(BF),
            "wvT": np.ascontiguousarray(Wv[KF * g:KF * (g + 1), :].T).astype(BF),
            "woT": np.ascontiguousarray(wo_g.T).astype(BF),
            "cost": cos128,
            "sint": sin128,
            "rotT": rotT,
            "maskd": maskb,
        }
        in_maps.append(im)

    trace = bool(int(os.environ.get("KERNEL_TRACE", "0")))
    res = bass_utils.run_bass_kernel_spmd(
        nc, in_maps, core_ids=list(range(8)), trace=trace)
    _CACHE["last_result"] = res

    out = np.zeros((B, S, H), np.float32)
    for core in range(8):
        g, b = core // 2, core % 2
        out[b] += np.asarray(res.results[core]["out"], np.float32)
    return out
